# revision 18
# baseline (speedup 1.0000x reference)
"""Trainium2 Bass kernel for nn_CrossAttention_12953621365289.

Self-attention where q=k=v share one projection (faithful to the reference's
to_q-overwrite bug).  Full inputs in, full output out; internally sharded
across 8 NeuronCores as (batch, query-half):

    core c -> batch b = c//2, query half = c%2

Each core receives x[b]^T with its query half rotated to the front (attention
is permutation-invariant over keys, so key order doesn't matter), computes
qkv for all 2048 tokens of its batch, runs attention for its 1024 queries over
all 16 heads, and applies the output projection for its 1024 tokens.  No
cross-core collectives are needed.

Math trick: scores = q @ q^T is symmetric, so the kernel computes score tiles
directly in transposed orientation [keys, queries] (keys on partitions),
which lets attn @ v run without any transposes: oT[d, i] accumulates
matmul(lhsT=v_aug[j_block], rhs=e[j_block, i]).  v_aug carries a ones column
so the softmax denominator Z falls out of the same accumulation (row 64).
Softmax skips max-subtraction (logits are small: diag ~8) and normalizes the
output instead of the weights: o[:, i] /= Z_i.

All matmul operands are float32r (fp32 data, relaxed-precision PE mode,
1 cycle/row at N>=256 vs 4 for fp32; measured ~1e-4 relative error).
"""

import sys

if '/opt/trn_rl_repo' not in sys.path:
    sys.path.insert(0, '/opt/trn_rl_repo')

import numpy as np

import concourse.bass as bass
import concourse.tile as tile
from concourse import bacc, mybir
from concourse.bass_utils import run_bass_kernel_spmd

F32 = mybir.dt.float32
F32R = mybir.dt.float32r
AF = mybir.ActivationFunctionType

B, N, D = 4, 2048, 1024
H, DH = 16, 64
NQ = N // 2          # queries per core
SCALE = DH ** -0.5
N_CORES = 8

_CACHE = {}


def _build_program(reps=1, vaug_mode="immid", a_copy_engine="vector"):
    nc = bacc.Bacc(name="cross_attn")

    xT_ext = nc.declare_dram_parameter("xT", [D, N], F32, isOutput=False)
    wq_ext = nc.declare_dram_parameter("w_qkv", [D, D], F32, isOutput=False)
    wo_ext = nc.declare_dram_parameter("w_out", [D, D], F32, isOutput=False)
    bo_ext = nc.declare_dram_parameter("b_out", [1, D], F32, isOutput=False)
    y_ext = nc.declare_dram_parameter("y", [NQ, D], F32, isOutput=True)

    KB = D // 128     # 8 d_model partition blocks
    TB = N // 128     # 16 token partition blocks
    VW = DH + 1       # 65: v columns + ones column

    with tile.TileContext(nc) as tc:
        with tc.tile_pool(name="persist", bufs=1) as persist, \
             tc.tile_pool(name="dram", bufs=1, space="DRAM") as dramp:
            # qkv^T: [channel, token] layout, channel-major head order
            qkvT = [persist.tile([128, N], F32R, tag=f"qkvT{k}",
                                 name=f"qkvT{k}") for k in range(KB)]
            # v in natural layout [token, head*65] with a ones column per head
            vaug = [persist.tile([128, H * VW], F32R, tag=f"vaug{t}",
                                 name=f"vaug{t}") for t in range(TB)]
            bias_sb = persist.tile([128, D], F32, tag="bias")
            ones_f32 = persist.tile([128, 64], F32, tag="onesf")
            ident_f32 = persist.tile([128, 128], F32, tag="identf")
            ident = persist.tile([128, 128], F32R, tag="ident")
            # DRAM scratch for the softmax-denominator broadcast bounce
            zscr = dramp.tile([H, NQ], F32, tag="zscr")

            nc.vector.memset(ones_f32[:], 1.0)
            from concourse.masks import make_identity
            make_identity(nc, ident_f32[:])
            nc.vector.tensor_copy(out=ident[:], in_=ident_f32[:])
            # bias broadcast to all partitions (DRAM-source stride-0 DMA)
            nc.gpsimd.dma_start(
                out=bias_sb[:], in_=bo_ext[:].partition_broadcast(128))
            # ones columns of v_aug
            for t in range(TB):
                va3 = vaug[t][:, :].rearrange("p (h w) -> p h w", h=H)
                nc.vector.tensor_copy(out=va3[:, :, DH], in_=ones_f32[:, 0:H])

          # reps>1 repeats the whole body inside one NEFF (benchmark builds)
            for _rep in range(reps):
                # ------------- Phase A: qkvT = (x @ W_qkv)^T ---------------
                # v_aug (natural layout) is derived from qkvT with PE
                # transpose-mode (1.5 cyc/row fp32r), much cheaper than a
                # second projection pass.
                with tc.tile_pool(name="wq", bufs=1) as wqp, \
                     tc.tile_pool(name="xtq", bufs=2) as xtp, \
                     tc.tile_pool(name="psA", bufs=4, space="PSUM") as psA, \
                     tc.tile_pool(name="psT", bufs=3, space="PSUM") as psT:
                    wqt = [wqp.tile([128, D], F32R, tag=f"wq{k}",
                                    name=f"wq{k}") for k in range(KB)]
                    for k in range(KB):
                        nc.gpsimd.dma_start(
                            out=wqt[k][:], in_=wq_ext[k * 128:(k + 1) * 128, :])
                    for tq in range(4):     # token quarters (512 each)
                        ts = slice(tq * 512, (tq + 1) * 512)
                        xt = xtp.tile([128, KB, 512], F32R, tag="xtq")
                        for k in range(KB):
                            nc.gpsimd.dma_start(
                                out=xt[:, k, :],
                                in_=xT_ext[k * 128:(k + 1) * 128, ts])
                        for cb in range(KB):
                            ps = psA.tile([128, 512], F32, tag="ps")
                            for k in range(KB):
                                nc.tensor.matmul(
                                    out=ps[:],
                                    lhsT=wqt[k][:, cb * 128:(cb + 1) * 128],
                                    rhs=xt[:, k, :],
                                    start=(k == 0), stop=(k == KB - 1))
                            (nc.vector.tensor_copy(out=qkvT[cb][:, ts], in_=ps[:])
                             if a_copy_engine == "vector" else
                             nc.scalar.copy(out=qkvT[cb][:, ts], in_=ps[:]))
                        if vaug_mode == "proj":
                            for tb in range(4):
                                t = tq * 4 + tb
                                for c5 in range(2):
                                    ps = psA.tile([128, 512], F32, tag="ps")
                                    for k in range(KB):
                                        nc.tensor.matmul(
                                            out=ps[:],
                                            lhsT=xt[:, k,
                                                    tb * 128:(tb + 1) * 128],
                                            rhs=wqt[k][:,
                                                       c5 * 512:(c5 + 1) * 512],
                                            start=(k == 0), stop=(k == KB - 1))
                                    va3 = vaug[t][:, :].rearrange(
                                        "p (h w) -> p h w", h=H)
                                    ps3 = ps[:].rearrange(
                                        "p (h d) -> p h d", h=8)
                                    nc.vector.tensor_copy(
                                        out=va3[:, c5 * 8:(c5 + 1) * 8, 0:DH],
                                        in_=ps3[:])
                    if vaug_mode in ("tmode", "immid"):
                        # all transposes after A1, t-outer so vaug[0] is
                        # ready first for phase B
                        for t in range(TB):
                            for cb in range(KB):
                                if vaug_mode == "tmode":
                                    trp = psT.tile([128, 128], F32R, tag="trp")
                                    nc.tensor.transpose(
                                        out=trp[:],
                                        in_=qkvT[cb][:, t * 128:(t + 1) * 128],
                                        identity=ident[:])
                                else:
                                    trp = psT.tile([128, 128], F32, tag="trp")
                                    nc.tensor.matmul(
                                        out=trp[:],
                                        lhsT=qkvT[cb][:, t * 128:(t + 1) * 128],
                                        rhs=ident[:],
                                        start=True, stop=True)
                                va3 = vaug[t][:, :].rearrange(
                                    "p (h w) -> p h w", h=H)
                                nc.vector.tensor_copy(
                                    out=va3[:, 2 * cb:2 * cb + 2, 0:DH],
                                    in_=trp[:].rearrange(
                                        "p (h d) -> p h d", h=2))

                # ------------- Phase B: attention per head -----------------
                with tc.tile_pool(name="ohT", bufs=1) as ohp:
                    ohT = [ohp.tile([128, NQ], F32R, tag=f"ohT{k}",
                                    name=f"ohT{k}") for k in range(KB)]
                    with tc.tile_pool(name="e", bufs=4) as ep, \
                         tc.tile_pool(name="mS", bufs=2, space="PSUM") as psS, \
                         tc.tile_pool(name="mO", bufs=2, space="PSUM") as psO, \
                         tc.tile_pool(name="misc", bufs=2) as mp:
                        for h in range(H):
                            pb, r0 = h // 2, (h % 2) * 64
                            qh = qkvT[pb][r0:r0 + 64, :]        # [64, 2048]
                            oT = psO.tile([VW, NQ], F32, tag="oT")
                            for j in range(TB):
                                sp = psS.tile([128, NQ], F32, tag="s")
                                for i5 in range(2):
                                    nc.tensor.matmul(
                                        out=sp[:, i5 * 512:(i5 + 1) * 512],
                                        lhsT=qh[:, j * 128:(j + 1) * 128],
                                        rhs=qh[:, i5 * 512:(i5 + 1) * 512],
                                        start=True, stop=True)
                                eb = ep.tile([128, NQ], F32R, tag="e")
                                nc.scalar.activation(
                                    out=eb[:], in_=sp[:], func=AF.Exp,
                                    scale=SCALE)
                                for i5 in range(2):
                                    nc.tensor.matmul(
                                        out=oT[:, i5 * 512:(i5 + 1) * 512],
                                        lhsT=vaug[j][:, h * VW:(h + 1) * VW],
                                        rhs=eb[:, i5 * 512:(i5 + 1) * 512],
                                        start=(j == 0), stop=(j == TB - 1))
                            # normalize: ohT[h] = oT[0:64] / Z  (Z = row 64)
                            # 1/Z broadcast to 64 partitions via DRAM bounce
                            # (stride-0 partition reads are DRAM-source only)
                            zr = mp.tile([1, NQ], F32, tag="zr")
                            nc.vector.reciprocal(out=zr[:], in_=oT[64:65, :])
                            nc.sync.dma_start(
                                out=zscr[h:h + 1, :], in_=zr[:])
                            rb = mp.tile([64, NQ], F32, tag="rb")
                            nc.gpsimd.dma_start(
                                out=rb[:],
                                in_=zscr[h:h + 1, :].partition_broadcast(64))
                            nc.vector.tensor_mul(
                                out=ohT[pb][r0:r0 + 64, :],
                                in0=oT[0:64, :], in1=rb[:])

                    # ------------- Phase C: y = ohT^T @ W_out + b ----------
                    with tc.tile_pool(name="wo", bufs=1) as wop, \
                         tc.tile_pool(name="yp", bufs=3) as yp, \
                         tc.tile_pool(name="psY", bufs=4, space="PSUM") as psY:
                        wot = [wop.tile([128, D], F32R, tag=f"wo{k}",
                                        name=f"wo{k}") for k in range(KB)]
                        for k in range(KB):
                            nc.gpsimd.dma_start(
                                out=wot[k][:],
                                in_=wo_ext[k * 128:(k + 1) * 128, :])
                        for tb in range(NQ // 128):
                            for n5 in range(2):
                                ps = psY.tile([128, 512], F32, tag="y")
                                for k in range(KB):
                                    nc.tensor.matmul(
                                        out=ps[:],
                                        lhsT=ohT[k][:, tb * 128:(tb + 1) * 128],
                                        rhs=wot[k][:, n5 * 512:(n5 + 1) * 512],
                                        start=(k == 0), stop=(k == KB - 1))
                                ysb = yp.tile([128, 512], F32, tag="ysb")
                                nc.vector.tensor_add(
                                    out=ysb[:], in0=ps[:],
                                    in1=bias_sb[:, n5 * 512:(n5 + 1) * 512])
                                nc.sync.dma_start(
                                    out=y_ext[tb * 128:(tb + 1) * 128,
                                              n5 * 512:(n5 + 1) * 512],
                                    in_=ysb[:])

    nc.finalize()
    return nc


class _Runner:
    """Caches the finalized Bass program and a jitted shard_map executable so
    repeated kernel() calls skip rebuild/retrace, and so execution can be
    benchmarked with device-resident inputs."""

    def __init__(self, reps=1, vaug_mode="immid", a_copy_engine="vector"):
        import jax
        from jax.sharding import Mesh, PartitionSpec
        from jax.experimental.shard_map import shard_map
        from concourse import mybir as _mybir
        from concourse.bass2jax import (
            _bass_exec_p, install_neuronx_cc_hook, partition_id_tensor)

        install_neuronx_cc_hook()
        nc = _build_program(reps=reps, vaug_mode=vaug_mode, a_copy_engine=a_copy_engine)
        self.nc = nc

        in_names, out_names, out_avals = [], [], []
        partition_name = (nc.partition_id_tensor.name
                          if nc.partition_id_tensor else None)
        for alloc in nc.m.functions[0].allocations:
            if not isinstance(alloc, _mybir.MemoryLocationSet):
                continue
            name = alloc.memorylocations[0].name
            if alloc.kind == "ExternalInput":
                if name != partition_name:
                    in_names.append(name)
            elif alloc.kind == "ExternalOutput":
                out_names.append(name)
                out_avals.append(jax.core.ShapedArray(
                    tuple(alloc.tensor_shape), _mybir.dt.np(alloc.dtype)))
        self.in_names = list(in_names)
        self.out_names = out_names
        self.out_avals = out_avals
        all_in_names = in_names + out_names
        if partition_name is not None:
            all_in_names = all_in_names + [partition_name]

        def _body(*args):
            operands = list(args)
            if partition_name is not None:
                operands.append(partition_id_tensor())
            outs = _bass_exec_p.bind(
                *operands,
                out_avals=tuple(out_avals),
                in_names=tuple(all_in_names),
                out_names=tuple(out_names),
                lowering_input_output_aliases=(),
                sim_require_finite=True,
                sim_require_nnan=True,
                nc=nc,
            )
            return tuple(outs)

        self._body = _body

        devices = jax.devices()[:N_CORES]
        mesh = Mesh(np.asarray(devices), ("core",))
        self.mesh = mesh
        n_in = len(in_names) + len(out_names)
        self.sharded = jax.jit(shard_map(
            _body, mesh=mesh,
            in_specs=(PartitionSpec("core"),) * n_in,
            out_specs=(PartitionSpec("core"),) * len(out_names),
            check_rep=False))
        self.zero_outs = [
            np.zeros((N_CORES * a.shape[0], *a.shape[1:]), a.dtype)
            for a in out_avals]

    def run_concat(self, concat_inputs):
        """concat_inputs: list matching in_names, each [8*dim0, ...]."""
        return self.sharded(*concat_inputs, *self.zero_outs)


def _get_runner():
    if "runner" not in _CACHE:
        _CACHE["runner"] = _Runner()
    return _CACHE["runner"]


def _shard_inputs(x, W_qkv, W_out, b_out):
    """Build the concatenated per-core input arrays (order = in_names)."""
    xts = []
    for c in range(N_CORES):
        b, half = c // 2, c % 2
        xt = np.ascontiguousarray(x[b].T)          # [D, N]
        if half:
            xt = np.concatenate([xt[:, NQ:], xt[:, :NQ]], axis=1)
        xts.append(xt)
    by_name = {
        "xT": np.concatenate(xts, axis=0),
        "w_qkv": np.tile(W_qkv, (N_CORES, 1)),
        "w_out": np.tile(W_out, (N_CORES, 1)),
        "b_out": np.tile(b_out, (N_CORES, 1)),
    }
    return by_name


def kernel(x, W_qkv, W_out, b_out):
    x = np.asarray(x, dtype=np.float32)
    W_qkv = np.asarray(W_qkv, dtype=np.float32)
    W_out = np.asarray(W_out, dtype=np.float32)
    b_out = np.asarray(b_out, dtype=np.float32).reshape(1, D)

    runner = _get_runner()
    by_name = _shard_inputs(x, W_qkv, W_out, b_out)
    concat_in = [by_name[n] for n in runner.in_names]
    outs = runner.run_concat(concat_in)

    y_all = np.asarray(outs[runner.out_names.index("y")])
    y_all = y_all.reshape(N_CORES, NQ, D)
    out = np.empty((B, N, D), np.float32)
    for c in range(N_CORES):
        b, half = c // 2, c % 2
        out[b, half * NQ:(half + 1) * NQ, :] = y_all[c]
    return out


# revision 21
# speedup vs baseline: 2.5869x; 2.5869x over previous
"""Trainium2 Bass kernel for nn_CrossAttention_12953621365289.

Self-attention where q=k=v share one projection (faithful to the reference's
to_q-overwrite bug).  Full inputs in, full output out; internally sharded
across 8 NeuronCores as (batch, query-half):

    core c -> batch b = c//2, query half = c%2

Each core receives x[b]^T with its query half rotated to the front (attention
is permutation-invariant over keys, so key order doesn't matter), computes
qkv for all 2048 tokens of its batch, runs attention for its 1024 queries over
all 16 heads, and applies the output projection for its 1024 tokens.  No
cross-core collectives are needed.

Math trick: scores = q @ q^T is symmetric, so the kernel computes score tiles
directly in transposed orientation [keys, queries] (keys on partitions),
which lets attn @ v run without any transposes: oT[d, i] accumulates
matmul(lhsT=v_aug[j_block], rhs=e[j_block, i]).  v_aug carries a ones column
so the softmax denominator Z falls out of the same accumulation (row 64).
Softmax skips max-subtraction (logits are small: diag ~8) and normalizes the
output instead of the weights: o[:, i] /= Z_i.

All matmul operands are float32r (fp32 data, relaxed-precision PE mode,
1 cycle/row at N>=256 vs 4 for fp32; measured ~1e-4 relative error).
"""

import sys

if '/opt/trn_rl_repo' not in sys.path:
    sys.path.insert(0, '/opt/trn_rl_repo')

import numpy as np

import concourse.bass as bass
import concourse.tile as tile
from concourse import bacc, mybir
from concourse.bass_utils import run_bass_kernel_spmd

F32 = mybir.dt.float32
F32R = mybir.dt.float32r
AF = mybir.ActivationFunctionType

B, N, D = 4, 2048, 1024
H, DH = 16, 64
NQ = N // 2          # queries per core
SCALE = DH ** -0.5
N_CORES = 8

_CACHE = {}


def _build_program(reps=1, vaug_mode="immid", a_copy_engine="vector", ebufs=4, mbufs=2):
    nc = bacc.Bacc(name="cross_attn")

    xT_ext = nc.declare_dram_parameter("xT", [D, N], F32, isOutput=False)
    wq_ext = nc.declare_dram_parameter("w_qkv", [D, D], F32, isOutput=False)
    wo_ext = nc.declare_dram_parameter("w_out", [D, D], F32, isOutput=False)
    bo_ext = nc.declare_dram_parameter("b_out", [1, D], F32, isOutput=False)
    y_ext = nc.declare_dram_parameter("y", [NQ, D], F32, isOutput=True)

    KB = D // 128     # 8 d_model partition blocks
    TB = N // 128     # 16 token partition blocks
    VW = DH + 1       # 65: v columns + ones column

    with tile.TileContext(nc) as tc:
        with tc.tile_pool(name="persist", bufs=1) as persist, \
             tc.tile_pool(name="dram", bufs=1, space="DRAM") as dramp:
            # qkv^T: [channel, token] layout, channel-major head order
            qkvT = [persist.tile([128, N], F32R, tag=f"qkvT{k}",
                                 name=f"qkvT{k}") for k in range(KB)]
            # v in natural layout [token, head*65] with a ones column per head
            vaug = [persist.tile([128, H * VW], F32R, tag=f"vaug{t}",
                                 name=f"vaug{t}") for t in range(TB)]
            bias_sb = persist.tile([128, D], F32, tag="bias")
            ones_f32 = persist.tile([128, 64], F32, tag="onesf")
            ident_f32 = persist.tile([128, 128], F32, tag="identf")
            ident = persist.tile([128, 128], F32R, tag="ident")
            ident2 = persist.tile([128, 256], F32R, tag="ident2")
            # DRAM scratch for the softmax-denominator broadcast bounce
            zscr = dramp.tile([H, NQ], F32, tag="zscr")

            nc.vector.memset(ones_f32[:], 1.0)
            from concourse.masks import make_identity
            make_identity(nc, ident_f32[:])
            nc.vector.tensor_copy(out=ident[:], in_=ident_f32[:])
            nc.vector.tensor_copy(out=ident2[:, 0:128], in_=ident_f32[:])
            nc.vector.tensor_copy(out=ident2[:, 128:256], in_=ident_f32[:])
            # bias broadcast to all partitions (DRAM-source stride-0 DMA)
            nc.gpsimd.dma_start(
                out=bias_sb[:], in_=bo_ext[:].partition_broadcast(128))
            # ones columns of v_aug
            for t in range(TB):
                va3 = vaug[t][:, :].rearrange("p (h w) -> p h w", h=H)
                nc.vector.tensor_copy(out=va3[:, :, DH], in_=ones_f32[:, 0:H])

          # reps>1 repeats the whole body inside one NEFF (benchmark builds)
            for _rep in range(reps):
                # ------------- Phase A: qkvT = (x @ W_qkv)^T ---------------
                # v_aug (natural layout) is derived from qkvT with identity
                # matmuls on the PE ("immid", ~27us) — cheaper than a second
                # projection pass ("proj", ~55us) and much faster on real HW
                # than PE transpose-mode ("tmode", which measures ~5-10x its
                # modeled cost and serializes the MM pipeline).
                with tc.tile_pool(name="wq", bufs=1) as wqp, \
                     tc.tile_pool(name="xtq", bufs=2) as xtp, \
                     tc.tile_pool(name="psA", bufs=4, space="PSUM") as psA, \
                     tc.tile_pool(name="psT", bufs=3, space="PSUM") as psT:
                    wqt = [wqp.tile([128, D], F32R, tag=f"wq{k}",
                                    name=f"wq{k}") for k in range(KB)]
                    for k in range(KB):
                        nc.gpsimd.dma_start(
                            out=wqt[k][:], in_=wq_ext[k * 128:(k + 1) * 128, :])
                    for tq in range(4):     # token quarters (512 each)
                        ts = slice(tq * 512, (tq + 1) * 512)
                        xt = xtp.tile([128, KB, 512], F32R, tag="xtq")
                        for k in range(KB):
                            nc.gpsimd.dma_start(
                                out=xt[:, k, :],
                                in_=xT_ext[k * 128:(k + 1) * 128, ts])
                        for cb in range(KB):
                            ps = psA.tile([128, 512], F32, tag="ps")
                            for k in range(KB):
                                nc.tensor.matmul(
                                    out=ps[:],
                                    lhsT=wqt[k][:, cb * 128:(cb + 1) * 128],
                                    rhs=xt[:, k, :],
                                    start=(k == 0), stop=(k == KB - 1))
                            (nc.vector.tensor_copy(out=qkvT[cb][:, ts], in_=ps[:])
                             if a_copy_engine == "vector" else
                             nc.scalar.copy(out=qkvT[cb][:, ts], in_=ps[:]))
                        if vaug_mode == "proj":
                            for tb in range(4):
                                t = tq * 4 + tb
                                for c5 in range(2):
                                    ps = psA.tile([128, 512], F32, tag="ps")
                                    for k in range(KB):
                                        nc.tensor.matmul(
                                            out=ps[:],
                                            lhsT=xt[:, k,
                                                    tb * 128:(tb + 1) * 128],
                                            rhs=wqt[k][:,
                                                       c5 * 512:(c5 + 1) * 512],
                                            start=(k == 0), stop=(k == KB - 1))
                                    va3 = vaug[t][:, :].rearrange(
                                        "p (h w) -> p h w", h=H)
                                    ps3 = ps[:].rearrange(
                                        "p (h d) -> p h d", h=8)
                                    nc.vector.tensor_copy(
                                        out=va3[:, c5 * 8:(c5 + 1) * 8, 0:DH],
                                        in_=ps3[:])
                    if vaug_mode in ("tmode", "immid", "immid2"):
                        # all transposes after A1, t-outer so vaug[0] is
                        # ready first for phase B
                        for t in range(TB):
                            for cb in range(KB):
                                if vaug_mode == "tmode":
                                    trp = psT.tile([128, 128], F32R, tag="trp")
                                    nc.tensor.transpose(
                                        out=trp[:],
                                        in_=qkvT[cb][:, t * 128:(t + 1) * 128],
                                        identity=ident[:])
                                elif vaug_mode == "immid":
                                    trp = psT.tile([128, 128], F32, tag="trp")
                                    nc.tensor.matmul(
                                        out=trp[:],
                                        lhsT=qkvT[cb][:, t * 128:(t + 1) * 128],
                                        rhs=ident[:],
                                        start=True, stop=True)
                                else:
                                    trp2 = psT.tile([128, 256], F32, tag="trp")
                                    nc.tensor.matmul(
                                        out=trp2[:],
                                        lhsT=qkvT[cb][:, t * 128:(t + 1) * 128],
                                        rhs=ident2[:],
                                        start=True, stop=True)
                                    trp = trp2[:, 0:128]
                                va3 = vaug[t][:, :].rearrange(
                                    "p (h w) -> p h w", h=H)
                                trp_ap = trp if vaug_mode == "immid2" \
                                    else trp[:]
                                nc.vector.tensor_copy(
                                    out=va3[:, 2 * cb:2 * cb + 2, 0:DH],
                                    in_=trp_ap.rearrange(
                                        "p (h d) -> p h d", h=2))

                # ------------- Phase B: attention per head -----------------
                with tc.tile_pool(name="ohT", bufs=1) as ohp:
                    ohT = [ohp.tile([128, NQ], F32R, tag=f"ohT{k}",
                                    name=f"ohT{k}") for k in range(KB)]
                    with tc.tile_pool(name="e", bufs=ebufs) as ep, \
                         tc.tile_pool(name="mS", bufs=2, space="PSUM") as psS, \
                         tc.tile_pool(name="mO", bufs=2, space="PSUM") as psO, \
                         tc.tile_pool(name="misc", bufs=mbufs) as mp:
                        for h in range(H):
                            pb, r0 = h // 2, (h % 2) * 64
                            qh = qkvT[pb][r0:r0 + 64, :]        # [64, 2048]
                            oT = psO.tile([VW, NQ], F32, tag="oT")
                            for j in range(TB):
                                sp = psS.tile([128, NQ], F32, tag="s")
                                for i5 in range(2):
                                    nc.tensor.matmul(
                                        out=sp[:, i5 * 512:(i5 + 1) * 512],
                                        lhsT=qh[:, j * 128:(j + 1) * 128],
                                        rhs=qh[:, i5 * 512:(i5 + 1) * 512],
                                        start=True, stop=True)
                                eb = ep.tile([128, NQ], F32R, tag="e")
                                nc.scalar.activation(
                                    out=eb[:], in_=sp[:], func=AF.Exp,
                                    scale=SCALE)
                                for i5 in range(2):
                                    nc.tensor.matmul(
                                        out=oT[:, i5 * 512:(i5 + 1) * 512],
                                        lhsT=vaug[j][:, h * VW:(h + 1) * VW],
                                        rhs=eb[:, i5 * 512:(i5 + 1) * 512],
                                        start=(j == 0), stop=(j == TB - 1))
                            # normalize: ohT[h] = oT[0:64] / Z  (Z = row 64)
                            # 1/Z broadcast to 64 partitions via DRAM bounce
                            # (stride-0 partition reads are DRAM-source only)
                            zr = mp.tile([1, NQ], F32, tag="zr")
                            nc.vector.reciprocal(out=zr[:], in_=oT[64:65, :])
                            nc.sync.dma_start(
                                out=zscr[h:h + 1, :], in_=zr[:])
                            rb = mp.tile([64, NQ], F32, tag="rb")
                            nc.gpsimd.dma_start(
                                out=rb[:],
                                in_=zscr[h:h + 1, :].partition_broadcast(64))
                            nc.vector.tensor_mul(
                                out=ohT[pb][r0:r0 + 64, :],
                                in0=oT[0:64, :], in1=rb[:])

                    # ------------- Phase C: y = ohT^T @ W_out + b ----------
                    with tc.tile_pool(name="wo", bufs=1) as wop, \
                         tc.tile_pool(name="yp", bufs=3) as yp, \
                         tc.tile_pool(name="psY", bufs=4, space="PSUM") as psY:
                        wot = [wop.tile([128, D], F32R, tag=f"wo{k}",
                                        name=f"wo{k}") for k in range(KB)]
                        for k in range(KB):
                            nc.gpsimd.dma_start(
                                out=wot[k][:],
                                in_=wo_ext[k * 128:(k + 1) * 128, :])
                        for tb in range(NQ // 128):
                            for n5 in range(2):
                                ps = psY.tile([128, 512], F32, tag="y")
                                for k in range(KB):
                                    nc.tensor.matmul(
                                        out=ps[:],
                                        lhsT=ohT[k][:, tb * 128:(tb + 1) * 128],
                                        rhs=wot[k][:, n5 * 512:(n5 + 1) * 512],
                                        start=(k == 0), stop=(k == KB - 1))
                                ysb = yp.tile([128, 512], F32, tag="ysb")
                                nc.vector.tensor_add(
                                    out=ysb[:], in0=ps[:],
                                    in1=bias_sb[:, n5 * 512:(n5 + 1) * 512])
                                nc.sync.dma_start(
                                    out=y_ext[tb * 128:(tb + 1) * 128,
                                              n5 * 512:(n5 + 1) * 512],
                                    in_=ysb[:])

    nc.finalize()
    return nc


class _Runner:
    """Caches the finalized Bass program and a jitted shard_map executable so
    repeated kernel() calls skip rebuild/retrace, and so execution can be
    benchmarked with device-resident inputs."""

    def __init__(self, reps=1, vaug_mode="immid", a_copy_engine="vector", ebufs=4, mbufs=2):
        import jax
        from jax.sharding import Mesh, PartitionSpec
        from jax.experimental.shard_map import shard_map
        from concourse import mybir as _mybir
        from concourse.bass2jax import (
            _bass_exec_p, install_neuronx_cc_hook, partition_id_tensor)

        install_neuronx_cc_hook()
        nc = _build_program(reps=reps, vaug_mode=vaug_mode, a_copy_engine=a_copy_engine, ebufs=ebufs, mbufs=mbufs)
        self.nc = nc

        in_names, out_names, out_avals = [], [], []
        partition_name = (nc.partition_id_tensor.name
                          if nc.partition_id_tensor else None)
        for alloc in nc.m.functions[0].allocations:
            if not isinstance(alloc, _mybir.MemoryLocationSet):
                continue
            name = alloc.memorylocations[0].name
            if alloc.kind == "ExternalInput":
                if name != partition_name:
                    in_names.append(name)
            elif alloc.kind == "ExternalOutput":
                out_names.append(name)
                out_avals.append(jax.core.ShapedArray(
                    tuple(alloc.tensor_shape), _mybir.dt.np(alloc.dtype)))
        self.in_names = list(in_names)
        self.out_names = out_names
        self.out_avals = out_avals
        all_in_names = in_names + out_names
        if partition_name is not None:
            all_in_names = all_in_names + [partition_name]

        def _body(*args):
            operands = list(args)
            if partition_name is not None:
                operands.append(partition_id_tensor())
            outs = _bass_exec_p.bind(
                *operands,
                out_avals=tuple(out_avals),
                in_names=tuple(all_in_names),
                out_names=tuple(out_names),
                lowering_input_output_aliases=(),
                sim_require_finite=True,
                sim_require_nnan=True,
                nc=nc,
            )
            return tuple(outs)

        self._body = _body

        devices = jax.devices()[:N_CORES]
        mesh = Mesh(np.asarray(devices), ("core",))
        self.mesh = mesh
        n_in = len(in_names) + len(out_names)
        self.sharded = jax.jit(shard_map(
            _body, mesh=mesh,
            in_specs=(PartitionSpec("core"),) * n_in,
            out_specs=(PartitionSpec("core"),) * len(out_names),
            check_rep=False))
        self.zero_outs = [
            np.zeros((N_CORES * a.shape[0], *a.shape[1:]), a.dtype)
            for a in out_avals]

    def run_concat(self, concat_inputs):
        """concat_inputs: list matching in_names, each [8*dim0, ...]."""
        return self.sharded(*concat_inputs, *self.zero_outs)


def _get_runner():
    if "runner" not in _CACHE:
        _CACHE["runner"] = _Runner()
    return _CACHE["runner"]


def _shard_inputs(x, W_qkv, W_out, b_out):
    """Build the concatenated per-core input arrays (order = in_names)."""
    xts = []
    for c in range(N_CORES):
        b, half = c // 2, c % 2
        xt = np.ascontiguousarray(x[b].T)          # [D, N]
        if half:
            xt = np.concatenate([xt[:, NQ:], xt[:, :NQ]], axis=1)
        xts.append(xt)
    by_name = {
        "xT": np.concatenate(xts, axis=0),
        "w_qkv": np.tile(W_qkv, (N_CORES, 1)),
        "w_out": np.tile(W_out, (N_CORES, 1)),
        "b_out": np.tile(b_out, (N_CORES, 1)),
    }
    return by_name


def kernel(x, W_qkv, W_out, b_out):
    x = np.asarray(x, dtype=np.float32)
    W_qkv = np.asarray(W_qkv, dtype=np.float32)
    W_out = np.asarray(W_out, dtype=np.float32)
    b_out = np.asarray(b_out, dtype=np.float32).reshape(1, D)

    runner = _get_runner()
    by_name = _shard_inputs(x, W_qkv, W_out, b_out)
    concat_in = [by_name[n] for n in runner.in_names]
    outs = runner.run_concat(concat_in)

    y_all = np.asarray(outs[runner.out_names.index("y")])
    y_all = y_all.reshape(N_CORES, NQ, D)
    out = np.empty((B, N, D), np.float32)
    for c in range(N_CORES):
        b, half = c // 2, c % 2
        out[b, half * NQ:(half + 1) * NQ, :] = y_all[c]
    return out
